# revision 12
# baseline (speedup 1.0000x reference)
"""DeepseekMoE layer on 8 Trainium2 NeuronCores (Bass/Tile, expert-parallel).

Sharding (per the expert-parallel hint):
  - 16 routed experts -> 2 per core ("slot 0" = the 8 largest-load experts,
    "slot 1" = the 8 smallest, paired big+small per core). Token dispatch
    (all-to-all) is emulated at the sharding layer: the host computes the
    discrete top-4 routing, gathers each expert's tokens into a compact
    transposed batch, and scatter-adds the compact expert outputs back into
    the full output ("combine").
  - Shared expert is tensor-parallel over its intermediate dim (2816/8 = 352
    columns per core, zero-padded to 384 = 3 x 128); the 8 partial outputs
    are summed on gather.
  - Gate (softmax + renormalized top-4 combine weights) is computed ON
    DEVICE per dispatched slot directly from the gathered activations; the
    host only supplies the discrete 0/1 top-4 mask rows (routing decision).
    Gate weights and mask columns are permuted per slot so the slot's own
    expert sits in column 0.

All FLOPs that produce output values run on device.  Matmuls run in bf16
(1 cycle/row on the PE, same as f32r, but half the HBM/SBUF traffic — the
f32 version of this kernel was DMA-bound at ~88% DMA occupancy).  PSUM
accumulation is fp32.

A single DMA queue sustains only ~200 GB/s (~8 SDMA engines), so the
routed-expert weight stream (the dominant traffic, needing ~280 GB/s
during routed phases) is split across two queues: even m-tiles on the
sync engine's queue, odd m-tiles on gpsimd's.  Activations/gate inputs
ride the scalar engine's queue, issued ahead of any scalar compute.

Weights are host-packed into stationary-tile-major layout ([m-tile,
partition, k-tile, col]) so each m-column's whole contraction loads as one
multi-KB-descriptor DMA.  Per-expert capacities are exact (C0 = max
slot-0 load, C1 = max slot-1 load) instead of one global capacity.
"""

import math
import os
import numpy as np
import ml_dtypes

H = 2048          # hidden size
E = 16            # routed experts
TOPK = 4
I = 1408          # routed expert intermediate
ISH = 2816        # shared expert intermediate
T = 1024          # tokens
P = 128
NCORES = 8
EPC = 2           # experts per core
ISS = ISH // NCORES                  # 352 shared columns per core
ISSP = 384                           # padded to 3 full 128-tiles
KH = H // P                          # 16 k-tiles over H
MI = I // P                          # 11 m-tiles over I
MH = H // P                          # 16 m-tiles over H
KI = I // P                          # 11 k-tiles over I
KS = ISSP // P                       # 3 k-tiles over padded shared slice

BF16 = ml_dtypes.bfloat16

_NC_CACHE = {}
LAST_RESULTS = None  # BassKernelResults of the most recent run (for test.py)


def _chunks(C, step):
    out = []
    off = 0
    while off < C:
        sz = min(step, C - off)
        out.append((off, sz))
        off += sz
    return out


def _pack_st(w, KT, MT):
    """[KT*P, MT*P] -> [MT*P, KT*P] tile-major stationary pack.

    packed[m*P + p, k*P + c] = w[k*P + p, m*P + c], so the device loads
    rows [m*P, (m+1)*P) as one [P, KT*P] block whose column-slice k is the
    stationary tile for (k, m).
    """
    return np.ascontiguousarray(
        w.reshape(KT, P, MT, P).transpose(2, 1, 0, 3).reshape(MT * P, KT * P))


def _build(C0, C1):
    import concourse.bacc as bacc
    import concourse.mybir as mybir
    import concourse.tile as tile
    from concourse.masks import make_identity

    f32 = mybir.dt.float32
    bf16 = mybir.dt.bfloat16
    SILU = mybir.ActivationFunctionType.Silu
    EXP = mybir.ActivationFunctionType.Exp
    X = mybir.AxisListType.X

    CS = [C0, C1]
    CT = C0 + C1
    NCH = [math.ceil(c / P) for c in CS]

    nc = bacc.Bacc("TRN2", target_bir_lowering=False, debug=False)

    XQ = 4                                   # xg0 load split (k-tile groups)
    xg_h = [nc.dram_tensor(f"xg{j}", [P, KH * CS[j]], bf16, kind="ExternalInput")
            for j in range(EPC)]
    mk_h = [nc.dram_tensor(f"mk{j}", [P, NCH[j] * E], f32, kind="ExternalInput")
            for j in range(EPC)]
    gwt_h = [nc.dram_tensor(f"gwt{j}", [P, KH * E], bf16, kind="ExternalInput")
             for j in range(EPC)]
    xt_h = nc.dram_tensor("xt", [P, KH * T], bf16, kind="ExternalInput")
    wg_h = [nc.dram_tensor(f"wg{j}", [I, H], bf16, kind="ExternalInput") for j in range(EPC)]
    wu_h = [nc.dram_tensor(f"wu{j}", [I, H], bf16, kind="ExternalInput") for j in range(EPC)]
    wd_h = [nc.dram_tensor(f"wd{j}", [H, I], bf16, kind="ExternalInput") for j in range(EPC)]
    swg_h = nc.dram_tensor("swg", [ISSP, H], bf16, kind="ExternalInput")
    swu_h = nc.dram_tensor("swu", [ISSP, H], bf16, kind="ExternalInput")
    swd_h = nc.dram_tensor("swd", [H, ISSP], bf16, kind="ExternalInput")
    zt_h = nc.dram_tensor("zt", [H, CT], bf16, kind="ExternalOutput")
    st_h = nc.dram_tensor("st", [H, T], bf16, kind="ExternalOutput")

    with tile.TileContext(nc) as tc:
        with (
            tc.tile_pool(name="resident", bufs=1) as res_pool,
            tc.tile_pool(name="xgp", bufs=1) as xg_pool,
            tc.tile_pool(name="acts", bufs=1) as act_pool,
            tc.tile_pool(name="wst_s", bufs=3) as wst_s,
            tc.tile_pool(name="wst_g", bufs=3) as wst_g,
            tc.tile_pool(name="dst_s", bufs=2) as dst_s,
            tc.tile_pool(name="dst_g", bufs=2) as dst_g,
            tc.tile_pool(name="sstream", bufs=2) as sst_pool,
            tc.tile_pool(name="small", bufs=2) as small_pool,
            tc.tile_pool(name="stage", bufs=3) as stage_pool,
            tc.tile_pool(name="ps", bufs=1, space="PSUM") as ps_pool,
        ):
            # ---------------- resident tiles / loads ----------------
            # identity + zbias first: gpsimd/vector compute before any DMA
            # issue so the gate's transposes never wait on them.
            ident = res_pool.tile([P, P], f32, name="ident", tag="ident")
            make_identity(nc, ident[:])
            zbias = res_pool.tile([P, 1], f32, name="zbias", tag="zbias")
            nc.vector.memset(zbias[:], 0.0)

            # scalar queue: all activation-side inputs, issued before any
            # scalar compute (exp/silu) appears in the scalar stream.
            # xg0 loads in k-tile groups (separate tiles) so the very first
            # upgate matmuls start as soon as group 0 lands.
            KG = KH // XQ
            xg0q = [xg_pool.tile([P, KG * CS[0]], bf16, name=f"xg0q{q}", tag=f"xg0q{q}")
                    for q in range(XQ)]
            for q in range(XQ):
                nc.scalar.dma_start(
                    xg0q[q][:], xg_h[0][:, q * KG * CS[0]:(q + 1) * KG * CS[0]])
            xgb1 = xg_pool.tile([P, KH * CS[1]], bf16, name="xgb1", tag="xgb1")
            nc.scalar.dma_start(xgb1[:], xg_h[1][:])
            gwtb = [res_pool.tile([P, KH * E], bf16, name=f"gwtb{j}", tag=f"gwtb{j}")
                    for j in range(EPC)]
            nc.scalar.dma_start(gwtb[0][:], gwt_h[0][:])
            nc.scalar.dma_start(gwtb[1][:], gwt_h[1][:])
            mkb = [res_pool.tile([P, NCH[j] * E], f32, name=f"mkb{j}", tag=f"mkb{j}")
                   for j in range(EPC)]
            nc.scalar.dma_start(mkb[0][:], mk_h[0][:])
            nc.scalar.dma_start(mkb[1][:], mk_h[1][:])
            xtb = res_pool.tile([P, KH * T], bf16, name="xtb", tag="xtb")
            for q in range(4):
                nc.scalar.dma_start(xtb[:, q * 4 * T:(q + 1) * 4 * T],
                                    xt_h[:, q * 4 * T:(q + 1) * 4 * T])
            xt_t = [xtb[:, k * T:(k + 1) * T] for k in range(KH)]
            # per-slot xg k-tile views
            xg_sl = [
                [xg0q[k // KG][:, (k % KG) * CS[0]:(k % KG + 1) * CS[0]] for k in range(KH)],
                [xgb1[:, k * CS[1]:(k + 1) * CS[1]] for k in range(KH)],
            ]

            wb = [res_pool.tile([P, CS[j]], f32, name=f"wb{j}", tag=f"wb{j}")
                  for j in range(EPC)]
            wcol = [[res_pool.tile([P, 1], f32, name=f"wcol{j}_{ch}", tag=f"wcol{j}_{ch}")
                     for ch in range(NCH[j])] for j in range(EPC)]
            a_t = [[act_pool.tile([P, CS[j]], bf16, name=f"a{j}_{m}", tag=f"a{j}_{m}")
                    for m in range(MI)] for j in range(EPC)]
            sg_t = [act_pool.tile([P, T], bf16, name=f"sg{m}", tag="sgtmp", bufs=2)
                    for m in range(KS)]
            as_t = [act_pool.tile([P, T], bf16, name=f"as{m}", tag=f"as{m}")
                    for m in range(KS)]

            # ---------------- emission sections ----------------
            def emit_gate_logits(j):
                # per-slot combine weights straight from the gathered
                # activations: logits with tokens stationary -> [csz, E]
                # token-major (no transpose); own expert = column 0.
                C = CS[j]
                lg = ps_pool.tile([P, NCH[j] * E], f32, name=f"lg{j}", tag="A1", bufs=4)
                for ch in range(NCH[j]):
                    coff = ch * P
                    csz = min(P, C - coff)
                    for k in range(KH):
                        nc.tensor.matmul(
                            lg[:csz, ch * E:(ch + 1) * E],
                            lhsT=xg_sl[j][k][:, coff:coff + csz],
                            rhs=gwtb[j][:, k * E:(k + 1) * E],
                            start=(k == 0), stop=(k == KH - 1),
                        )
                for ch in range(NCH[j]):
                    coff = ch * P
                    csz = min(P, C - coff)
                    sc = small_pool.tile([P, E], f32, name=f"sc{j}_{ch}", tag="sc")
                    nc.scalar.activation(sc[:csz], lg[:csz, ch * E:(ch + 1) * E],
                                         EXP, bias=zbias[:csz])
                    mskd = small_pool.tile([P, E], f32, name=f"mskd{j}_{ch}", tag="mskd")
                    nc.vector.tensor_mul(out=mskd[:csz], in0=sc[:csz],
                                         in1=mkb[j][:csz, ch * E:(ch + 1) * E])
                    ssum = small_pool.tile([P, 1], f32, name=f"ssum{j}_{ch}", tag="ssum")
                    nc.vector.reduce_sum(ssum[:csz], mskd[:csz], axis=X)
                    rsum = small_pool.tile([P, 1], f32, name=f"rsum{j}_{ch}", tag="rsum")
                    nc.vector.reciprocal(rsum[:csz], ssum[:csz])
                    nc.vector.tensor_scalar_mul(wcol[j][ch][:csz], mskd[:csz, 0:1],
                                                rsum[:csz, :1])

            def emit_gate_broadcast(j):
                # partition-broadcast of the per-slot weights: emitted well
                # after the logits so the PE never waits on the DVE chain.
                C = CS[j]
                for ch in range(NCH[j]):
                    coff = ch * P
                    csz = min(P, C - coff)
                    wbps = ps_pool.tile([P, P], f32, name=f"wbps{j}_{ch}", tag="A1", bufs=4)
                    nc.tensor.transpose(
                        out=wbps[:, :csz],
                        in_=wcol[j][ch][:csz, :1].to_broadcast([csz, P]),
                        identity=ident[0:csz, 0:csz],
                    )
                    nc.vector.tensor_copy(wb[j][:, coff:coff + csz], wbps[:, :csz])

            def emit_upgate(j):
                C = CS[j]
                xg_t = xg_sl[j]
                g_t = [act_pool.tile([P, C], bf16, name=f"g{j}_{m}", tag="gtmp", bufs=3)
                       for m in range(MI)]
                for m in range(MI):
                    # wg/wu of each m ride different queues (halves the
                    # per-queue burst for the tile the PE is waiting on)
                    pa, ea, pb, eb = ((wst_s, nc.sync, wst_g, nc.gpsimd) if m % 2 == 0
                                      else (wst_g, nc.gpsimd, wst_s, nc.sync))
                    wgb = pa.tile([P, KH * P], bf16, name=f"wgb{j}_{m}", tag="wblk", bufs=4)
                    ea.dma_start(wgb[:], wg_h[j][m * P:(m + 1) * P, :])
                    wub = pb.tile([P, KH * P], bf16, name=f"wub{j}_{m}", tag="wblk", bufs=4)
                    eb.dma_start(wub[:], wu_h[j][m * P:(m + 1) * P, :])
                    for (coff, csz) in _chunks(C, 512):
                        psg = ps_pool.tile([P, csz], f32, name=f"psg{j}_{m}_{coff}", tag="A1", bufs=4)
                        for k in range(KH):
                            nc.tensor.matmul(psg[:], lhsT=wgb[:, k * P:(k + 1) * P],
                                             rhs=xg_t[k][:, coff:coff + csz],
                                             start=(k == 0), stop=(k == KH - 1))
                        nc.scalar.activation(g_t[m][:, coff:coff + csz], psg[:], SILU, bias=zbias[:])
                        psu = ps_pool.tile([P, csz], f32, name=f"psu{j}_{m}_{coff}", tag="A1", bufs=4)
                        for k in range(KH):
                            nc.tensor.matmul(psu[:], lhsT=wub[:, k * P:(k + 1) * P],
                                             rhs=xg_t[k][:, coff:coff + csz],
                                             start=(k == 0), stop=(k == KH - 1))
                        # a = silu(g) * u straight out of PSUM, rounded to bf16
                        nc.vector.tensor_mul(out=a_t[j][m][:, coff:coff + csz],
                                             in0=g_t[m][:, coff:coff + csz], in1=psu[:])

            def emit_down(j, interleave=None):
                C = CS[j]
                off_j = 0 if j == 0 else C0
                for m in range(MH):
                    if interleave is not None and m % 2 == 0:
                        interleave(m // 2)
                    pool, eng = (dst_s, nc.sync) if m % 2 == 0 else (dst_g, nc.gpsimd)
                    wdb = pool.tile([P, KI * P], bf16, name=f"wdb{j}_{m}", tag="wdb", bufs=5)
                    eng.dma_start(wdb[:], wd_h[j][m * P:(m + 1) * P, :])
                    for (coff, csz) in _chunks(C, 512):
                        psz = ps_pool.tile([P, csz], f32, name=f"psz{j}_{m}_{coff}", tag="A1", bufs=4)
                        for k in range(KI):
                            nc.tensor.matmul(psz[:], lhsT=wdb[:, k * P:(k + 1) * P],
                                             rhs=a_t[j][k][:, coff:coff + csz],
                                             start=(k == 0), stop=(k == KI - 1))
                        zst = stage_pool.tile([P, csz], bf16, name=f"zst{j}_{m}_{coff}", tag="zst", bufs=3)
                        # combine-weight scaling fused into the eviction
                        nc.vector.tensor_mul(out=zst[:], in0=wb[j][:, coff:coff + csz], in1=psz[:])
                        nc.gpsimd.dma_start(
                            zt_h[m * P:(m + 1) * P, off_j + coff:off_j + coff + csz], zst[:])

            def emit_shared_ug(mi):
                sgb = sst_pool.tile([P, KH * P], bf16, name=f"sgb{mi}", tag="ssb", bufs=2)
                nc.sync.dma_start(sgb[:], swg_h[mi * P:(mi + 1) * P, :])
                psgs = ps_pool.tile([P, T], f32, name=f"psgs{mi}", tag="B1", bufs=2)
                for k in range(KH):
                    for (noff, nsz) in _chunks(T, 512):
                        nc.tensor.matmul(psgs[:, noff:noff + nsz],
                                         lhsT=sgb[:, k * P:(k + 1) * P],
                                         rhs=xt_t[k][:, noff:noff + nsz],
                                         start=(k == 0), stop=(k == KH - 1))
                nc.scalar.activation(sg_t[mi][:], psgs[:], SILU, bias=zbias[:])
                sub = sst_pool.tile([P, KH * P], bf16, name=f"sub{mi}", tag="ssb", bufs=2)
                nc.sync.dma_start(sub[:], swu_h[mi * P:(mi + 1) * P, :])
                psus = ps_pool.tile([P, T], f32, name=f"psus{mi}", tag="B1", bufs=2)
                for k in range(KH):
                    for (noff, nsz) in _chunks(T, 512):
                        nc.tensor.matmul(psus[:, noff:noff + nsz],
                                         lhsT=sub[:, k * P:(k + 1) * P],
                                         rhs=xt_t[k][:, noff:noff + nsz],
                                         start=(k == 0), stop=(k == KH - 1))
                nc.vector.tensor_mul(out=as_t[mi][:], in0=sg_t[mi][:], in1=psus[:])

            def emit_shared_down(ms):
                for m in ms:
                    sdb = sst_pool.tile([P, KS * P], bf16, name=f"sdb{m}", tag="sdb", bufs=2)
                    nc.sync.dma_start(sdb[:], swd_h[m * P:(m + 1) * P, :])
                    psys = ps_pool.tile([P, T], f32, name=f"psys{m}", tag="B1", bufs=2)
                    for ki in range(KS):
                        for (noff, nsz) in _chunks(T, 512):
                            nc.tensor.matmul(psys[:, noff:noff + nsz],
                                             lhsT=sdb[:, ki * P:(ki + 1) * P],
                                             rhs=as_t[ki][:, noff:noff + nsz],
                                             start=(ki == 0), stop=(ki == KS - 1))
                    sstg = stage_pool.tile([P, T], bf16, name=f"sstg{m}", tag="sstage", bufs=3)
                    nc.scalar.copy(sstg[:], psys[:])
                    # st writes alternate queues (a single queue can't keep
                    # up with the tail eviction rate)
                    eng = nc.scalar if m % 2 == 0 else nc.gpsimd
                    eng.dma_start(st_h[m * P:(m + 1) * P, :], sstg[:])

            emit_upgate(0)
            emit_gate_logits(0)
            emit_gate_logits(1)
            emit_gate_broadcast(0)
            emit_gate_broadcast(1)
            emit_shared_ug(0)
            emit_down(0)
            emit_upgate(1)
            emit_shared_ug(1)
            emit_down(1)
            emit_shared_ug(2)
            emit_shared_down(list(range(MH)))

    nc.compile()
    return nc


def _get_nc(C0, C1):
    key = (C0, C1)
    if key not in _NC_CACHE:
        _NC_CACHE[key] = _build(C0, C1)
    return _NC_CACHE[key]


def kernel(**inputs):
    global LAST_RESULTS
    from concourse.bass_utils import run_bass_kernel_spmd

    hs = np.asarray(inputs["hidden_states"], dtype=np.float32)
    gate_w = np.asarray(inputs["gate_w"], dtype=np.float32)
    w_gate = np.asarray(inputs["w_gate"], dtype=np.float32)
    w_up = np.asarray(inputs["w_up"], dtype=np.float32)
    w_down = np.asarray(inputs["w_down"], dtype=np.float32)
    sw_gate = np.asarray(inputs["sw_gate"], dtype=np.float32)
    sw_up = np.asarray(inputs["sw_up"], dtype=np.float32)
    sw_down = np.asarray(inputs["sw_down"], dtype=np.float32)

    orig_shape = hs.shape
    x = hs.reshape(-1, H)
    assert x.shape[0] == T

    # ---- host: discrete routing only (top-4 selection + dispatch tables) ----
    logits = x @ gate_w.T
    smax = logits.max(axis=-1, keepdims=True)
    sc = np.exp(logits - smax)
    sc /= sc.sum(axis=-1, keepdims=True)
    order = np.argsort(-sc, axis=-1, kind="stable")[:, :TOPK]
    mask = np.zeros((T, E), dtype=np.float32)
    mask[np.arange(T)[:, None], order] = 1.0
    tok_lists = [np.nonzero(mask[:, e])[0].astype(np.int64) for e in range(E)]
    counts = np.array([len(tk) for tk in tok_lists])

    # slot 0 = the 8 largest-load experts, slot 1 = the 8 smallest
    rank = np.argsort(-counts, kind="stable")
    slot_experts = [rank[:NCORES], rank[NCORES:][::-1]]  # pair big with small
    C0 = int(counts[slot_experts[0]].max())
    C1 = int(counts[slot_experts[1]].max())

    nc = _get_nc(C0, C1)
    CS = [C0, C1]
    NCH = [math.ceil(c / P) for c in CS]

    xb = x.astype(BF16)
    xTb = np.ascontiguousarray(xb.T)                     # [H, T] bf16
    # xt packed: xtb[p, k*T + t] = x[t, k*P + p]
    xtb = np.ascontiguousarray(
        xTb.reshape(KH, P, T).transpose(1, 0, 2).reshape(P, KH * T))
    gate_wT = gate_w.T.astype(BF16)                      # [H, E] bf16

    # shared slices, zero-padded to 384 and tile-major packed (per core below)
    def pad_cols(w, newc):
        out = np.zeros((w.shape[0], newc), dtype=w.dtype)
        out[:, :w.shape[1]] = w
        return out

    def pad_rows(w, newr):
        out = np.zeros((newr, w.shape[1]), dtype=w.dtype)
        out[:w.shape[0], :] = w
        return out

    swg_b = sw_gate.astype(BF16)
    swu_b = sw_up.astype(BF16)
    swd_b = sw_down.astype(BF16)
    wg_b = w_gate.astype(BF16)
    wu_b = w_up.astype(BF16)
    wd_b = w_down.astype(BF16)

    in_maps = []
    core_experts = []
    for c in range(NCORES):
        es = [int(slot_experts[j][c]) for j in range(EPC)]
        core_experts.append(es)
        im = {
            "xt": xtb,
            "swg": _pack_st(pad_cols(swg_b[:, c * ISS:(c + 1) * ISS], ISSP), KH, KS),
            "swu": _pack_st(pad_cols(swu_b[:, c * ISS:(c + 1) * ISS], ISSP), KH, KS),
            "swd": _pack_st(pad_rows(swd_b[c * ISS:(c + 1) * ISS, :], ISSP), KS, MH),
        }
        for j, e in enumerate(es):
            tk = tok_lists[e]
            n = len(tk)
            C = CS[j]
            # gathered activations, tile-major: xg[p, k*C + s] = x[tok_s, k*P + p]
            xg = np.zeros((P, KH * C), dtype=BF16)
            g = xTb[:, tk].reshape(KH, P, n).transpose(1, 0, 2)  # [P, KH, n]
            xg.reshape(P, KH, C)[:, :, :n] = g
            im[f"xg{j}"] = xg
            # gate weights + mask rows share a per-slot permutation with the
            # slot's own expert in column 0 (softmax sums are perm-invariant)
            perm = [e] + [q for q in range(E) if q != e]
            gwtp = gate_wT[:, perm]                              # [H, E]
            im[f"gwt{j}"] = np.ascontiguousarray(
                gwtp.reshape(KH, P, E).transpose(1, 0, 2).reshape(P, KH * E))
            mkc = np.zeros((P, NCH[j] * E), dtype=np.float32)
            mrows = mask[tk][:, perm]                            # [n, E]
            for ch in range(NCH[j]):
                lo = ch * P
                sz = min(P, n - lo)
                if sz > 0:
                    mkc[:sz, ch * E:(ch + 1) * E] = mrows[lo:lo + sz]
            im[f"mk{j}"] = mkc
            im[f"wg{j}"] = _pack_st(wg_b[e], KH, MI)
            im[f"wu{j}"] = _pack_st(wu_b[e], KH, MI)
            im[f"wd{j}"] = _pack_st(wd_b[e], KI, MH)
        in_maps.append(im)

    trace = bool(int(os.environ.get("BASSMOE_TRACE", "0")))
    kwargs = {}
    if trace:
        kwargs = dict(trace=True, tmpdir=os.environ.get("BASSMOE_TRACE_DIR") or None)
        tcores = os.environ.get("BASSMOE_TRACE_CORES")
        if tcores:
            kwargs["trace_cores"] = [int(x) for x in tcores.split(",")]
            kwargs["stitch_traces"] = False
    res = run_bass_kernel_spmd(nc, in_maps, core_ids=list(range(NCORES)), **kwargs)
    LAST_RESULTS = res

    # ---- host: unshard (scatter-add compact expert outputs + sum partials) ----
    y = np.zeros((T, H), dtype=np.float64)
    st_sum = np.zeros((H, T), dtype=np.float64)
    for c in range(NCORES):
        r = res.results[c]
        st_sum += np.asarray(r["st"], dtype=np.float32)
        zt = np.asarray(r["zt"], dtype=np.float32)
        for j in range(EPC):
            e = core_experts[c][j]
            tk = tok_lists[e]
            off = 0 if j == 0 else C0
            y[tk] += zt[:, off:off + len(tk)].T
    y += st_sum.T
    return y.astype(np.float32).reshape(orig_shape)
